# revision 9
# baseline (speedup 1.0000x reference)
"""Trainium2 Bass kernel for nn_KATrainPythiaModel (6-layer Pythia-style
transformer, B=2, S=1024, H=512, FF=2048, V=50304).

Key mathematical fact (verified to 5e-8 end-to-end): the ka_scan attention
variant collapses to rank-1 — A[i] = r_i * v[token0] with r_i == 1 up to
O(1e-7) (deviation only via the +1e-8 renormalization epsilon). So each
layer's attention output is a per-batch constant row:
    attn[b, s, :] = (LN(h)[b, 0, :] @ Wcomb + bcomb)   for all s,
with Wcomb = (dense_w @ Wv).T folded on host.

Distribution: 2048 token rows sharded 8 ways (256/core) + each core carries
its batch's token-0 row as a 257th column so layers need NO communication.
One AllGather of the final hidden states precedes a vocab-sharded LM head.
Matmuls run in bf16 (fp32 accumulate); residual stream/LN in fp32.
"""
import numpy as np
from contextlib import ExitStack

import ml_dtypes
import concourse.bass as bass
import concourse.bacc as bacc
import concourse.tile as tile
import concourse.mybir as mybir
import concourse.bass_utils as bass_utils
from concourse.masks import make_identity

L, H, NH, HD, FF, V = 6, 512, 8, 64, 2048, 50304
B, S = 2, 1024
NCORE = 8
TOK = (B * S) // NCORE          # 256 exclusive tokens per core
T = TOK + 1                     # + replicated token-0 column
VS = V // NCORE                 # 6288 vocab rows per core
EPS = 1e-5
BF = mybir.dt.bfloat16
F32 = mybir.dt.float32
NP_BF = ml_dtypes.bfloat16

_cache = {}


def _build_bass():
    nc = bacc.Bacc("TRN2", target_bir_lowering=False, debug=False,
                   num_devices=NCORE)

    h0 = nc.dram_tensor("h0", [T, H], F32, kind="ExternalInput")
    w1 = nc.dram_tensor("w1", [L, H, FF], BF, kind="ExternalInput")   # fc1_w.T (gains folded)
    w2 = nc.dram_tensor("w2", [L, FF, H], BF, kind="ExternalInput")   # fc2_w.T
    wc = nc.dram_tensor("wc", [L, H, H], BF, kind="ExternalInput")    # (dense@Wv).T
    b1 = nc.dram_tensor("b1", [128, L * 16], F32, kind="ExternalInput")
    b2 = nc.dram_tensor("b2", [128, L * 4], F32, kind="ExternalInput")
    bc = nc.dram_tensor("bc", [1, L * H], F32, kind="ExternalInput")
    wh = nc.dram_tensor("wh", [H, VS], BF, kind="ExternalInput")      # out_w slice .T
    lg = nc.dram_tensor("lg", [B * S, VS], F32, kind="ExternalOutput")

    cc_in = nc.dram_tensor("cc_in", [4, 128, TOK], BF, kind="Internal")
    cc_out = nc.dram_tensor("cc_out", [NCORE, 4, 128, TOK], BF,
                            kind="Internal", addr_space="Shared")

    with tile.TileContext(nc) as tc, ExitStack() as ctx:
        const = ctx.enter_context(tc.tile_pool(name="const", bufs=1))
        wpool = ctx.enter_context(tc.tile_pool(name="wts", bufs=2))
        hpool = ctx.enter_context(tc.tile_pool(name="hstate", bufs=1))
        apool = ctx.enter_context(tc.tile_pool(name="acts", bufs=2))
        spool = ctx.enter_context(tc.tile_pool(name="small", bufs=2))
        ppool = ctx.enter_context(tc.tile_pool(name="ps", bufs=6, space="PSUM"))
        hdpool = ctx.enter_context(tc.tile_pool(name="head", bufs=1))
        opool = ctx.enter_context(tc.tile_pool(name="outs", bufs=3))

        id_bf = const.tile([128, 128], BF)
        make_identity(nc, id_bf[:])
        id_f = const.tile([128, 128], F32)
        make_identity(nc, id_f[:])
        eps_t = const.tile([128, 1], F32)
        nc.vector.memset(eps_t[:], EPS)
        ones_bf = const.tile([1, 128], BF)
        nc.vector.memset(ones_bf[:], 1.0)

        b1t = const.tile([128, L * 16], F32)
        nc.sync.dma_start(b1t[:], b1.ap())
        b2t = const.tile([128, L * 4], F32)
        nc.sync.dma_start(b2t[:], b2.ap())
        bct = const.tile([1, L * H], F32)
        nc.sync.dma_start(bct[:], bc.ap())

        # LM-head weights resident (overlaps body compute)
        wht = [hdpool.tile([128, VS], BF, tag=f"wh{hc}", name=f"wh{hc}") for hc in range(4)]
        for hc in range(4):
            nc.sync.dma_start(wht[hc][:], wh.ap()[hc * 128:(hc + 1) * 128, :])

        # residual stream: 2x[128,512] + [1,512] (token0)
        hs = [hpool.tile([128, H], F32, tag="hA", name="hA"),
              hpool.tile([128, H], F32, tag="hB", name="hB"),
              hpool.tile([1, H], F32, tag="hC", name="hC")]
        nc.sync.dma_start(hs[0][:], h0.ap()[0:128, :])
        nc.sync.dma_start(hs[1][:], h0.ap()[128:256, :])
        nc.sync.dma_start(hs[2][:], h0.ap()[256:257, :])

        def layernorm_to_bf(xt, p, tag):
            """fp32 [p,512] -> bf16 normalized (no affine; folded on host)."""
            m = spool.tile([128, 1], F32, tag="lnm")
            nc.vector.reduce_sum(m[:p], xt[:p], axis=mybir.AxisListType.X)
            nc.scalar.mul(m[:p], m[:p], 1.0 / H)
            xm = apool.tile([128, H], F32, tag="lnxm")
            nc.vector.tensor_scalar_sub(xm[:p], xt[:p], m[:p])
            sq = apool.tile([128, H], F32, tag="lnsq")
            nc.scalar.square(sq[:p], xm[:p])
            v_ = spool.tile([128, 1], F32, tag="lnv")
            nc.vector.reduce_sum(v_[:p], sq[:p], axis=mybir.AxisListType.X)
            sd = spool.tile([128, 1], F32, tag="lnsd")
            nc.scalar.activation(sd[:p], v_[:p], mybir.ActivationFunctionType.Sqrt,
                                 bias=eps_t[:p], scale=1.0 / H)
            rs = spool.tile([128, 1], F32, tag="lnrs")
            nc.vector.reciprocal(rs[:p], sd[:p])
            xn = apool.tile([128, H], BF, tag=tag, name=tag)
            nc.vector.tensor_scalar_mul(xn[:p], xm[:p], rs[:p])
            return xn

        def transpose_x(xns, ncols):
            """[tok-major bf16 tiles] -> 4x [128h, ncols] bf16 (ncols=T or TOK)."""
            xT = [apool.tile([128, T], BF, tag=f"xT{hc}", name=f"xT{hc}") for hc in range(4)]
            for hc in range(4):
                sl = slice(hc * 128, (hc + 1) * 128)
                for tt in range(2):
                    pt = ppool.tile([128, 128], BF, tag="psb", bufs=2)
                    nc.tensor.transpose(pt[:], xns[tt][:, sl], id_bf[:])
                    nc.vector.tensor_copy(xT[hc][:, tt * 128:(tt + 1) * 128], pt[:])
                if ncols == T:
                    pt = ppool.tile([128, 128], BF, tag="psb", bufs=2)
                    nc.tensor.transpose(pt[:128, 0:1], xns[2][0:1, sl], id_bf[0:1, 0:1])
                    nc.vector.tensor_copy(xT[hc][:, 256:257], pt[:128, 0:1])
            return xT

        for l in range(L):
            # stream this layer's weights
            w1s = [wpool.tile([128, FF], BF, tag=f"w1_{hc}", name=f"w1_{hc}") for hc in range(4)]
            for hc in range(4):
                nc.sync.dma_start(w1s[hc][:], w1.ap()[l, hc * 128:(hc + 1) * 128, :])
            w2s = [wpool.tile([128, H], BF, tag=f"w2_{ft}", name=f"w2_{ft}", bufs=1) for ft in range(16)]
            for ft in range(16):
                nc.sync.dma_start(w2s[ft][:], w2.ap()[l, ft * 128:(ft + 1) * 128, :])
            wcs = [wpool.tile([128, H], BF, tag=f"wc_{hc}", name=f"wc_{hc}", bufs=1) for hc in range(4)]
            for hc in range(4):
                nc.sync.dma_start(wcs[hc][:], wc.ap()[l, hc * 128:(hc + 1) * 128, :])

            xn0 = layernorm_to_bf(hs[0], 128, "xnA")
            xn1 = layernorm_to_bf(hs[1], 128, "xnB")
            xn2 = layernorm_to_bf(hs[2], 1, "xnC")
            xT = transpose_x([xn0, xn1, xn2], T)

            # fc1 + gelu (y1 kept f-major bf16)
            g1 = [apool.tile([128, T], BF, tag=f"g1_{ft}", name=f"g1_{ft}") for ft in range(16)]
            for ft in range(16):
                py = ppool.tile([128, T], F32, tag="ps")
                for hc in range(4):
                    nc.tensor.matmul(py[:], w1s[hc][:, ft * 128:(ft + 1) * 128],
                                     xT[hc][:], start=(hc == 0), stop=(hc == 3))
                nc.scalar.activation(g1[ft][:], py[:],
                                     mybir.ActivationFunctionType.Gelu,
                                     bias=b1t[:, l * 16 + ft:l * 16 + ft + 1])

            # fc2 -> y2T (h-major, fp32)
            y2T = [apool.tile([128, T], F32, tag=f"y2T{ht}", name=f"y2T{ht}") for ht in range(4)]
            for ht in range(4):
                py = ppool.tile([128, T], F32, tag="ps")
                for ft in range(16):
                    nc.tensor.matmul(py[:], w2s[ft][:, ht * 128:(ht + 1) * 128],
                                     g1[ft][:], start=(ft == 0), stop=(ft == 15))
                nc.scalar.activation(y2T[ht][:], py[:],
                                     mybir.ActivationFunctionType.Identity,
                                     bias=b2t[:, l * 4 + ht:l * 4 + ht + 1])

            # attention contribution: arow = x0 @ Wcomb + bcomb  (x0 = col 256)
            arow = spool.tile([1, H], F32, tag="arow")
            for hp in range(4):
                pa = ppool.tile([128, 1], F32, tag="ps")
                for hc in range(4):
                    nc.tensor.matmul(pa[:], wcs[hc][:, hp * 128:(hp + 1) * 128],
                                     xT[hc][:, 256:257],
                                     start=(hc == 0), stop=(hc == 3))
                asb = spool.tile([128, 1], F32, tag="asb")
                nc.vector.tensor_copy(asb[:], pa[:])
                pr = ppool.tile([1, 128], F32, tag="ps")
                nc.tensor.transpose(pr[:], asb[:], id_f[:])
                nc.vector.tensor_add(arow[:, hp * 128:(hp + 1) * 128], pr[:],
                                     bct[:, l * H + hp * 128:l * H + (hp + 1) * 128])
            arow_bf = spool.tile([1, H], BF, tag="arowbf")
            nc.vector.tensor_copy(arow_bf[:], arow[:])
            pb = ppool.tile([128, H], F32, tag="ps")
            nc.tensor.matmul(pb[:], ones_bf[:], arow_bf[:], start=True, stop=True)

            # residual update: h += attn_bcast + y2
            for tt in range(2):
                nc.vector.tensor_add(hs[tt][:], hs[tt][:], pb[:])
                for ht in range(4):
                    sl = slice(ht * 128, (ht + 1) * 128)
                    pt = ppool.tile([128, 128], F32, tag="ps")
                    nc.tensor.transpose(pt[:], y2T[ht][:, tt * 128:(tt + 1) * 128],
                                        id_f[:])
                    nc.vector.tensor_add(hs[tt][:, sl], hs[tt][:, sl], pt[:])
            nc.vector.tensor_add(hs[2][:], hs[2][:], pb[0:1, :])
            for ht in range(4):
                sl = slice(ht * 128, (ht + 1) * 128)
                prow = ppool.tile([1, 128], F32, tag="ps")
                nc.tensor.transpose(prow[:], y2T[ht][:, 256:257], id_f[:])
                nc.vector.tensor_add(hs[2][:, sl], hs[2][:, sl], prow[:])

        # final LN + transpose own 256 tokens, AllGather
        fn0 = layernorm_to_bf(hs[0], 128, "xnA")
        fn1 = layernorm_to_bf(hs[1], 128, "xnB")
        fT = transpose_x([fn0, fn1, None], TOK)
        for hc in range(4):
            nc.sync.dma_start(cc_in.ap()[hc], fT[hc][:, 0:TOK])
        nc.gpsimd.collective_compute(
            "AllGather", mybir.AluOpType.bypass,
            replica_groups=[list(range(NCORE))],
            ins=[cc_in.ap()], outs=[cc_out.ap()],
        )
        hf = [hdpool.tile([128, B * S], BF, tag=f"hf{hc}", name=f"hf{hc}") for hc in range(4)]
        for hc in range(4):
            for r in range(NCORE):
                nc.sync.dma_start(hf[hc][:, r * TOK:(r + 1) * TOK],
                                  cc_out.ap()[r, hc])

        # LM head: 16 token tiles x 13 vocab chunks, K=4 accumulation
        vchunks = [(i * 512, 512) for i in range(12)] + [(6144, 144)]
        for ttile in range(16):
            tsl = slice(ttile * 128, (ttile + 1) * 128)
            for (voff, vw) in vchunks:
                ph = ppool.tile([128, 512], F32, tag="ps")
                for hc in range(4):
                    nc.tensor.matmul(ph[:, 0:vw], hf[hc][:, tsl],
                                     wht[hc][:, voff:voff + vw],
                                     start=(hc == 0), stop=(hc == 3))
                osb = opool.tile([128, 512], F32, tag="osb")
                nc.scalar.activation(osb[:, 0:vw], ph[:, 0:vw],
                                     mybir.ActivationFunctionType.Copy)
                nc.sync.dma_start(lg.ap()[tsl, voff:voff + vw], osb[:, 0:vw])

    nc.compile()
    return nc


def _prep_inputs(inputs):
    """Host-side sharding/folding. Returns per-core in_maps."""
    f32 = np.float32
    ids = np.asarray(inputs["input_ids"]).reshape(-1)          # [2048]
    emb = np.asarray(inputs["embed_table"], dtype=f32)
    h0_full = emb[ids]                                          # [2048, 512]
    ln_g = np.asarray(inputs["ln_g"], dtype=f32)
    ln_b = np.asarray(inputs["ln_b"], dtype=f32)
    qkv_w = np.asarray(inputs["qkv_w"], dtype=f32)
    qkv_b = np.asarray(inputs["qkv_b"], dtype=f32)
    dense_w = np.asarray(inputs["dense_w"], dtype=f32)
    dense_b = np.asarray(inputs["dense_b"], dtype=f32)
    fc1_w = np.asarray(inputs["fc1_w"], dtype=f32)
    fc1_b = np.asarray(inputs["fc1_b"], dtype=f32)
    fc2_w = np.asarray(inputs["fc2_w"], dtype=f32)
    fc2_b = np.asarray(inputs["fc2_b"], dtype=f32)
    fln_g = np.asarray(inputs["fln_g"], dtype=f32)
    fln_b = np.asarray(inputs["fln_b"], dtype=f32)
    out_w = np.asarray(inputs["out_w"], dtype=f32)

    w1t = np.empty((L, H, FF), NP_BF)
    w2t = np.empty((L, FF, H), NP_BF)
    wct = np.empty((L, H, H), NP_BF)
    b1p_all = np.empty((L, FF), f32)
    b2p_all = np.empty((L, H), f32)
    bc_all = np.empty((L, H), f32)
    for l in range(L):
        w1_eff = fc1_w[l] * ln_g[l][None, :]                    # [FF,H]
        w1t[l] = w1_eff.T.astype(NP_BF)
        b1p_all[l] = fc1_b[l] + ln_b[l] @ fc1_w[l].T
        w2t[l] = fc2_w[l].T.astype(NP_BF)                       # [FF,H]
        b2p_all[l] = fc2_b[l]
        wv = qkv_w[l, 2 * H:3 * H, :]                           # [512,512]
        bv = qkv_b[l, 2 * H:3 * H]
        wcomb = (dense_w[l] @ (wv * ln_g[l][None, :])).T        # [h,h']
        wct[l] = wcomb.astype(NP_BF)
        bc_all[l] = (ln_b[l] @ wv.T + bv) @ dense_w[l].T + dense_b[l]

    b1p = b1p_all.reshape(L, 16, 128).transpose(2, 0, 1).reshape(128, L * 16).copy()
    b2p = b2p_all.reshape(L, 4, 128).transpose(2, 0, 1).reshape(128, L * 4).copy()
    bcp = bc_all.reshape(1, L * H).copy()
    ow_eff = out_w * fln_g[None, :]                             # [V,H]
    vbias = fln_b @ out_w.T                                     # [V]

    in_maps = []
    for c in range(NCORE):
        tok0 = 0 if c < 4 else S
        h0c = np.concatenate([h0_full[c * TOK:(c + 1) * TOK],
                              h0_full[tok0:tok0 + 1]], axis=0)
        whc = np.ascontiguousarray(
            ow_eff[c * VS:(c + 1) * VS, :].T).astype(NP_BF)      # [H,VS]
        in_maps.append({
            "h0": np.ascontiguousarray(h0c, dtype=f32),
            "w1": w1t, "w2": w2t, "wc": wct,
            "b1": b1p, "b2": b2p, "bc": bcp,
            "wh": whc,
        })
    return in_maps, vbias


def kernel(**inputs):
    if "nc" not in _cache:
        _cache["nc"] = _build_bass()
    nc = _cache["nc"]
    in_maps, vbias = _prep_inputs(inputs)
    res = bass_utils.run_bass_kernel_spmd(
        nc, in_maps, core_ids=list(range(NCORE)))
    parts = [res.results[c]["lg"] for c in range(NCORE)]
    logits = np.concatenate(parts, axis=1).reshape(B, S, V)
    if np.any(vbias):
        logits = logits + vbias[None, None, :]
    return logits.astype(np.float32)
